# revision 1
# baseline (speedup 1.0000x reference)
"""MoE top-2 routing kernel for Trainium2, expert-parallel across 8 NeuronCores.

Strategy (per sharding_hint: expert-parallel, one expert per core):
  - Host computes the router's *discrete* top-2 choice in f32 numpy (selection
    verified identical to the jax reference; min prob gap between ranks 2/3 on
    this problem is ~1e-5, far above f32 rounding noise) and uses it only to
    build the token->expert dispatch (the "all-to-all"): tokens routed to
    expert c are gathered, transposed, and padded to a common capacity Cap.
  - Each core receives its expert's gathered tokens xgT [H, Cap] (f32), its
    expert's W1/W2 (tiled layout), and a column-rolled router matrix Wr (own
    expert in column 0). The device recomputes router logits in f32, derives
    the top-2 combine weight w for its own expert, computes
    y = (silu(x@W1 + b1) @ W2 + b2) * w entirely on-device, and writes
    yT [H, Cap] f32.
  - Host scatter-adds the per-core outputs back into token order.

Device per-chunk pipeline (Tc=512 tokens):
  router matmul (f32) -> top2 combine weight (DVE/ACT/GPSIMD) ->
  phase A: hT = silu(W1^T x + b1)   (f32r or bf16 matmuls, f32 psum)
  phase B: yT = W2^T hT accumulated over 4 i-groups
  -> scale by w, add b2, DMA out.

MM_DTYPE: "f32r" (fp32 storage, ~1.5e-4 matmul rel err, full PE rate at
N>=256) or "bf16" (~2.3e-3, half the DMA traffic).
"""

import numpy as np
import ml_dtypes

import concourse.bacc as bacc
import concourse.tile as tile
import concourse.mybir as mybir
import concourse.bass_isa as bass_isa
from concourse import bass_utils

BF16NP = ml_dtypes.bfloat16
F32 = mybir.dt.float32
F32R = mybir.dt.float32r
BF16 = mybir.dt.bfloat16
F16 = mybir.dt.float16
AF = mybir.ActivationFunctionType
ALU = mybir.AluOpType

B, S, H, I, E = 4, 2048, 1024, 4096, 8
T = B * S
TOP_K = 2
NCORES = 8
TC = 512            # token chunk
KH = H // 128       # 8  k-tiles over H (contraction of matmul 1 / router)
NI = I // 128       # 32 i-tiles over I
NH = H // 128       # 8  output h-tiles
GI = 8              # i-tiles per PSUM accumulation group in phase B
NEG = -1.0e30

MM_DTYPE = "f32r"   # "f32r" | "bf16"


def _chunks(cap):
    out, t0 = [], 0
    while t0 < cap:
        tw = min(TC, cap - t0)
        out.append((t0, tw))
        t0 += tw
    return out


def _build_nc(cap, mmdt=MM_DTYPE, reps=1, loop_n=None):
    f32r = mmdt == "f32r"
    WDT = {"f32r": F32R, "bf16": BF16, "f16": F16}[mmdt]
    # In f32r mode every matmul operand chain is *typed* f32r end-to-end
    # (dram -> sbuf -> matmul). No bitcasts: bitcast() clones the tensor
    # handle, which breaks Tile's dependency tracking (observed as
    # nondeterministic races).
    XDT = F32R if f32r else F32

    nc = bacc.Bacc(
        "TRN2",
        target_bir_lowering=False,
        debug=False,
        enable_asserts=False,
        num_devices=NCORES,
    )
    eye = nc.dram_tensor("eye", [128, 128], F32, kind="ExternalInput").ap()
    xg = nc.dram_tensor("xg", [KH, 128, cap], XDT, kind="ExternalInput").ap()
    w1 = nc.dram_tensor("w1", [NI, 128, KH * 128], WDT, kind="ExternalInput").ap()
    w2 = nc.dram_tensor("w2", [NI, 128, H], WDT, kind="ExternalInput").ap()
    wr = nc.dram_tensor("wr", [KH, 128, E], XDT, kind="ExternalInput").ap()
    b1r = nc.dram_tensor("b1r", [128, NI], F32, kind="ExternalInput").ap()
    b2r = nc.dram_tensor("b2r", [128, NH], F32, kind="ExternalInput").ap()
    yt = nc.dram_tensor("yt", [NH, 128, cap], F32, kind="ExternalOutput").ap()

    with tile.TileContext(nc) as tc:
        with (
            tc.tile_pool(name="consts", bufs=1) as cpool,
            tc.tile_pool(name="xf", bufs=2) as xf_pool,
            tc.tile_pool(name="w1p", bufs=3) as w1_pool,
            tc.tile_pool(name="w2p", bufs=10) as w2_pool,
            tc.tile_pool(name="hp", bufs=1) as h_pool,
            tc.tile_pool(name="yp", bufs=1 if f32r else 2) as y_pool,
            tc.tile_pool(name="rp", bufs=1) as r_pool,
            tc.tile_pool(name="wbp", bufs=2) as wb_pool,
            tc.tile_pool(name="php", bufs=2, space="PSUM") as ph_pool,
            tc.tile_pool(name="pyp", bufs=2, space="PSUM") as py_pool,
            tc.tile_pool(name="prp", bufs=2, space="PSUM") as pr_pool,
            tc.tile_pool(name="xbp", bufs=2) as xb_pool,
        ):
            # consts packed into one tile: [b1 | b2] (f32) + separate wr tile
            cw = NI + NH
            consts = cpool.tile([128, cw], F32)
            b1_sb = consts[:, 0:NI]
            b2_sb = consts[:, NI:cw]
            nc.sync.dma_start(b1_sb, b1r[:, :])
            nc.sync.dma_start(b2_sb, b2r[:, :])
            wr_sb = cpool.tile([128, KH * E], XDT)
            for k in range(KH):
                nc.sync.dma_start(wr_sb[:, k * E:(k + 1) * E], wr[k])
            eye_sb = cpool.tile([128, 128], F32)
            nc.sync.dma_start(eye_sb[:], eye[:, :])
            ones1 = cpool.tile([1, 128], F32)
            nc.vector.memset(ones1[:], 1.0)

            import contextlib
            loop_cm = (
                tc.For_i(0, loop_n, 1, hint_engines=(mybir.EngineType.PE,))
                if loop_n else contextlib.nullcontext()
            )
            with loop_cm:
                _emit_body(nc, tc, cap, reps, f32r, locals())

    nc.compile()
    return nc


def _emit_body(nc, tc, cap, reps, f32r, env):
    xg, w1, w2, yt = env["xg"], env["w1"], env["w2"], env["yt"]
    wr_sb, b1_sb, b2_sb = env["wr_sb"], env["b1_sb"], env["b2_sb"]
    eye_sb, ones1 = env["eye_sb"], env["ones1"]
    xf_pool, w1_pool, w2_pool = env["xf_pool"], env["w1_pool"], env["w2_pool"]
    h_pool, y_pool, r_pool = env["h_pool"], env["y_pool"], env["r_pool"]
    wb_pool, ph_pool, py_pool = env["wb_pool"], env["ph_pool"], env["py_pool"]
    pr_pool, xb_pool = env["pr_pool"], env["xb_pool"]
    XDT = F32R if f32r else F32
    WDT = env["WDT"]
    if True:
            for (t0, tw) in [c for _ in range(reps) for c in _chunks(cap)]:
                # ---- load x chunk (transposed: H on partitions) ----
                xf = xf_pool.tile([128, KH * TC], XDT, tag="xf")
                for k in range(KH):
                    nc.sync.dma_start(
                        xf[:, k * TC:k * TC + tw], xg[k][:, t0:t0 + tw]
                    )

                # ---- router: logitsT [E, tw] in f32 ----
                pl = pr_pool.tile([E, TC], F32, tag="pr")
                for k in range(KH):
                    nc.tensor.matmul(
                        pl[:, :tw],
                        wr_sb[:, k * E:(k + 1) * E],
                        xf[:, k * TC:k * TC + tw],
                        start=(k == 0),
                        stop=(k == KH - 1),
                    )
                # Per 128-token m-tile: PE-transpose logits to [128tok, E],
                # then top-2 + combine weight in token-on-partition layout
                # (pure DVE/ACT; no gpsimd). w = exp(l0-m1)/(1+exp(m2-m1)).
                mt = tw // 128
                r = r_pool.tile([128, 64], F32, tag="r")
                wq = r[:, 52:52 + 4]
                Ls = r_pool.tile([E, TC], F32, tag="Ls")
                nc.scalar.copy(Ls[:, :tw], pl[:, :tw])
                for m in range(mt):
                    ltp = pr_pool.tile([128, E], F32, tag="misc")
                    nc.tensor.transpose(
                        ltp[:, :], Ls[:, m * 128:(m + 1) * 128],
                        eye_sb[0:E, 0:E],
                    )
                    lt = r[:, m * 8:m * 8 + E]
                    nc.scalar.copy(lt, ltp[:, :])
                    m1 = r[:, 32 + m:33 + m]
                    nc.vector.reduce_max(m1, lt, axis=mybir.AxisListType.X)
                    ge = r[:, 56:56 + E]
                    nc.vector.tensor_scalar(ge, lt, m1, None, op0=ALU.is_ge)
                    nc.vector.tensor_scalar_mul(ge, ge, NEG)
                    nc.vector.tensor_tensor(ge, ge, lt, op=ALU.add)
                    m2 = r[:, 36 + m:37 + m]
                    nc.vector.reduce_max(m2, ge, axis=mybir.AxisListType.X)
                    m1n = r[:, 40 + m:41 + m]
                    nc.vector.tensor_scalar_mul(m1n, m1, -1.0)
                    e1 = r[:, 44 + m:45 + m]
                    nc.scalar.activation(e1, lt[:, 0:1], AF.Exp, bias=m1n)
                    e2 = r[:, 48 + m:49 + m]
                    nc.scalar.activation(e2, m2, AF.Exp, bias=m1n)
                    nc.vector.tensor_scalar_add(e2, e2, 1.0)
                    nc.vector.reciprocal(e2, e2)
                    nc.vector.tensor_tensor(wq[:, m:m + 1], e1, e2, op=ALU.mult)
                # transpose w columns to a row; outer-product with ones
                # broadcasts across partitions: wb[p, t] = w[t]
                wt = wb_pool.tile([1, TC], F32, tag="wt")
                for m in range(mt):
                    wtp = pr_pool.tile([1, 128], F32, tag="misc")
                    nc.tensor.transpose(
                        wtp[:, :], wq[:, m:m + 1], eye_sb[:, :]
                    )
                    nc.scalar.copy(wt[0:1, m * 128:(m + 1) * 128], wtp[:, :])
                wbp = pr_pool.tile([128, TC], F32, tag="misc")
                nc.tensor.matmul(wbp[:, :tw], ones1[:, :], wt[0:1, :tw])
                wb = wb_pool.tile([128, TC], F32, tag="wb")
                nc.scalar.copy(wb[:, :tw], wbp[:, :tw])

                # ---- phase A rhs: f32r direct, or bf16 cast ----
                if f32r:
                    xmm = xf
                else:
                    xmm = xb_pool.tile([128, KH * TC], WDT, tag="xb")
                    for k in range(KH):
                        nc.vector.tensor_copy(
                            xmm[:, k * TC:k * TC + tw], xf[:, k * TC:k * TC + tw]
                        )

                # ---- phase A: hT[i-tile] = silu(W1^T x + b1) ----
                h = h_pool.tile([128, NI * TC], F32R if f32r else WDT, tag="h")
                hmm = h
                for i in range(NI):
                    w1t = w1_pool.tile([128, KH * 128], WDT, tag="w1t")
                    nc.sync.dma_start(w1t[:], w1[i])
                    ph = ph_pool.tile([128, TC], F32, tag="ph")
                    for k in range(KH):
                        nc.tensor.matmul(
                            ph[:, :tw],
                            w1t[:, k * 128:(k + 1) * 128],
                            xmm[:, k * TC:k * TC + tw],
                            start=(k == 0),
                            stop=(k == KH - 1),
                        )
                    nc.scalar.activation(
                        h[:, i * TC:i * TC + tw], ph[:, :tw], AF.Silu,
                        bias=b1_sb[:, i:i + 1],
                    )

                # ---- phase B: yT += W2^T hT over i-groups ----
                y = y_pool.tile([128, NH * TC], F32, tag="y")
                for gi in range(NI // GI):
                    w2ts = []
                    for j in range(GI):
                        w2t = w2_pool.tile([128, H], WDT, tag="w2t")
                        nc.sync.dma_start(w2t[:], w2[gi * GI + j])
                        w2ts.append(w2t)
                    for hb in range(NH):
                        py = py_pool.tile([128, TC], F32, tag="py")
                        for j in range(GI):
                            i = gi * GI + j
                            nc.tensor.matmul(
                                py[:, :tw],
                                w2ts[j][:, hb * 128:(hb + 1) * 128],
                                hmm[:, i * TC:i * TC + tw],
                                start=(j == 0),
                                stop=(j == GI - 1),
                            )
                        ys = y[:, hb * TC:hb * TC + tw]
                        if gi == 0:
                            nc.scalar.activation(
                                ys, py[:, :tw], AF.Identity,
                                bias=b2_sb[:, hb:hb + 1],
                            )
                        else:
                            nc.vector.tensor_tensor(ys, ys, py[:, :tw], op=ALU.add)

                # ---- scale by combine weight, write out ----
                for hb in range(NH):
                    ys = y[:, hb * TC:hb * TC + tw]
                    nc.vector.tensor_tensor(ys, ys, wb[:, :tw], op=ALU.mult)
                    nc.sync.dma_start(yt[hb][:, t0:t0 + tw], ys)


def _route_host(xf, Wr):
    logits = xf @ Wr
    m = logits.max(-1, keepdims=True)
    e = np.exp(logits - m)
    probs = e / e.sum(-1, keepdims=True)
    return np.argsort(-probs, axis=-1, kind="stable")[:, :TOP_K]


def kernel_ex(x, Wr, W1, b1, W2, b2, trace=False, mmdt=MM_DTYPE):
    x = np.ascontiguousarray(np.asarray(x, dtype=np.float32))
    Wr = np.asarray(Wr, dtype=np.float32)
    W1 = np.asarray(W1, dtype=np.float32)
    b1 = np.asarray(b1, dtype=np.float32)
    W2 = np.asarray(W2, dtype=np.float32)
    b2 = np.asarray(b2, dtype=np.float32)

    xf = x.reshape(T, H)
    sel = _route_host(xf, Wr)

    idx = [np.nonzero((sel == c).any(-1))[0] for c in range(E)]
    cap = max(len(ix) for ix in idx)
    cap = max(256, -(-cap // 256) * 256)

    wnp = {"f32r": np.float32, "bf16": BF16NP, "f16": np.float16}[mmdt]
    in_maps = []
    for c in range(E):
        ix = idx[c]
        xgT = np.zeros((H, cap), np.float32)
        xgT[:, :len(ix)] = xf[ix].T
        roll = [c] + [e for e in range(E) if e != c]
        # w1 sbuf layout: [i-tile][p, k*128+f] = W1[k*128+p, i*128+f]
        w1r = np.ascontiguousarray(
            W1[c].reshape(KH, 128, NI, 128).transpose(2, 1, 0, 3)
            .reshape(NI, 128, KH * 128)
        ).astype(wnp)
        in_maps.append({
            "eye": np.eye(128, dtype=np.float32),
            "xg": np.ascontiguousarray(xgT.reshape(KH, 128, cap)),
            "w1": w1r,
            "w2": np.ascontiguousarray(W2[c].reshape(NI, 128, H)).astype(wnp),
            "wr": np.ascontiguousarray(Wr[:, roll].reshape(KH, 128, E)),
            "b1r": np.ascontiguousarray(b1[c].reshape(NI, 128).T),
            "b2r": np.ascontiguousarray(b2[c].reshape(NH, 128).T),
        })

    nc = _build_nc(cap, mmdt)
    try:
        res = bass_utils.run_bass_kernel_spmd(
            nc, in_maps, core_ids=list(range(NCORES)), trace=trace
        )
    except ModuleNotFoundError:
        # no axon NTFF profile hook in this container -> run untraced
        res = bass_utils.run_bass_kernel_spmd(
            nc, in_maps, core_ids=list(range(NCORES)), trace=False
        )

    out = np.zeros((T, H), np.float32)
    for c in range(E):
        ix = idx[c]
        yc = res.results[c]["yt"].reshape(H, cap)
        out[ix] += yc.T[:len(ix)]
    return out.reshape(B, S, H), res


def kernel(**inputs):
    out, _ = kernel_ex(**inputs)
    return out



# revision 2
# speedup vs baseline: 1.5769x; 1.5769x over previous
"""MoE top-2 routing kernel for Trainium2, expert-parallel across 8 NeuronCores.

Strategy (per sharding_hint: expert-parallel, one expert per core):
  - Host computes the router (softmax + top-2 + combine weights) in f32
    numpy (selection matches the jax reference; the router is 0.05% of
    total FLOPs) and builds the token->expert dispatch: tokens routed to
    expert c are gathered, transposed, padded to a common capacity Cap
    (max expert load rounded up to 128).
  - Each core receives its expert's gathered tokens xgT [H, Cap] (bf16),
    its expert's W1/W2 (bf16, matmul-tiled layout), the per-token combine
    weight pre-broadcast to [128, Cap] f32, and biases. The core computes
    yT = (silu(x@W1 + b1) @ W2 + b2) * w and writes yT [H, Cap] f32.
  - Host scatter-adds the per-core outputs back into token order.

Device structure (the key difference from the v1 baseline, which
re-streamed all 33.6MB of f32 weights from HBM for every 512-token
chunk, ~151MB of DMA vs ~490us of PE work):
  - W1 and W2 are bf16 and loaded into SBUF ONCE (64KB + 64KB per
    partition), then all token chunks stream through them.
  - Phase A per chunk: hT[i] = silu(W1[i]^T x + b1[i]), 32 i-tiles,
    PSUM-accumulated over 8 k-tiles.
  - Phase B per chunk: yT[hb] = sum_j W2[j][hb]^T hT[j], PSUM-accumulated
    over all 32 j-tiles in one accumulation group, then +b2, *w, DMA out.
  - All matmuls are bf16 (1 cycle/row on PE, same rate as f32r) with
    512-wide moving data; expected rel err ~2e-3 (tolerance 2e-2).
"""

import numpy as np
import ml_dtypes

import concourse.bacc as bacc
import concourse.tile as tile
import concourse.mybir as mybir
from concourse import bass_utils

BF16NP = ml_dtypes.bfloat16
F32 = mybir.dt.float32
BF16 = mybir.dt.bfloat16
AF = mybir.ActivationFunctionType
ALU = mybir.AluOpType

B, S, H, I, E = 4, 2048, 1024, 4096, 8
T = B * S
TOP_K = 2
NCORES = 8
TC = 512            # token chunk (= max f32 PSUM bank width)
KH = H // 128       # 8  k-tiles over H (contraction of matmul 1)
NI = I // 128       # 32 i-tiles over I
NH = H // 128       # 8  output h-tiles


def _chunks(cap):
    out, t0 = [], 0
    while t0 < cap:
        tw = min(TC, cap - t0)
        out.append((t0, tw))
        t0 += tw
    return out


def _build_nc(cap, loop_n=None):
    nc = bacc.Bacc(
        "TRN2",
        target_bir_lowering=False,
        debug=False,
        enable_asserts=False,
        num_devices=NCORES,
    )
    xg = nc.dram_tensor("xg", [KH, 128, cap], BF16, kind="ExternalInput").ap()
    w1 = nc.dram_tensor("w1", [NI, 128, KH * 128], BF16, kind="ExternalInput").ap()
    w2 = nc.dram_tensor("w2", [NI, 128, H], BF16, kind="ExternalInput").ap()
    wb = nc.dram_tensor("wb", [128, cap], F32, kind="ExternalInput").ap()
    b1r = nc.dram_tensor("b1r", [128, NI], F32, kind="ExternalInput").ap()
    b2r = nc.dram_tensor("b2r", [128, NH], F32, kind="ExternalInput").ap()
    yt = nc.dram_tensor("yt", [NH, 128, cap], F32, kind="ExternalOutput").ap()

    with tile.TileContext(nc) as tc:
        with (
            tc.tile_pool(name="consts", bufs=1) as cpool,
            tc.tile_pool(name="w1p", bufs=1) as w1_pool,
            tc.tile_pool(name="w2p", bufs=1) as w2_pool,
            tc.tile_pool(name="xsp", bufs=2) as xs_pool,
            tc.tile_pool(name="hp", bufs=1) as h_pool,
            tc.tile_pool(name="ysp", bufs=3) as ys_pool,
            tc.tile_pool(name="php", bufs=2, space="PSUM") as ph_pool,
            tc.tile_pool(name="pyp", bufs=2, space="PSUM") as py_pool,
        ):
            consts = cpool.tile([128, NI + NH], F32)
            b1_sb = consts[:, 0:NI]
            b2_sb = consts[:, NI:NI + NH]
            nc.sync.dma_start(b1_sb, b1r[:, :])
            nc.sync.dma_start(b2_sb, b2r[:, :])
            wb_sb = cpool.tile([128, cap], F32)
            nc.sync.dma_start(wb_sb[:], wb[:, :])

            import contextlib
            loop_cm = (
                tc.For_i(0, loop_n, 1, hint_engines=(mybir.EngineType.PE,))
                if loop_n else contextlib.nullcontext()
            )
            with loop_cm:
                _emit_body(nc, cap, locals())

    nc.compile()
    return nc


def _emit_body(nc, cap, env):
    xg, w1, w2, yt = env["xg"], env["w1"], env["w2"], env["yt"]
    b1_sb, b2_sb, wb_sb = env["b1_sb"], env["b2_sb"], env["wb_sb"]
    w1_pool, w2_pool = env["w1_pool"], env["w2_pool"]
    xs_pool, h_pool, ys_pool = env["xs_pool"], env["h_pool"], env["ys_pool"]
    ph_pool, py_pool = env["ph_pool"], env["py_pool"]

    WS = KH * 128  # 1024 cols per w1 i-tile / w2 j-tile

    # ---- resident weights: w1 then w2, with the first x chunk's DMA
    # interleaved right after w1[0] so PE can start almost immediately ----
    w1_sb = w1_pool.tile([128, NI * WS], BF16, tag="w1")
    w2_sb = w2_pool.tile([128, NI * H], BF16, tag="w2")
    nc.sync.dma_start(w1_sb[:, 0:WS], w1[0])

    chunks = _chunks(cap)
    t0, tw = chunks[0]
    xs0 = xs_pool.tile([128, KH * TC], BF16, tag="xs")
    for k in range(KH):
        nc.sync.dma_start(xs0[:, k * TC:k * TC + tw], xg[k][:, t0:t0 + tw])

    for i in range(1, NI):
        nc.sync.dma_start(w1_sb[:, i * WS:(i + 1) * WS], w1[i])
    for j in range(NI):
        nc.sync.dma_start(w2_sb[:, j * H:(j + 1) * H], w2[j])

    for ci, (t0, tw) in enumerate(chunks):
        if ci == 0:
            xs = xs0
        else:
            xs = xs_pool.tile([128, KH * TC], BF16, tag="xs")
            for k in range(KH):
                nc.sync.dma_start(xs[:, k * TC:k * TC + tw], xg[k][:, t0:t0 + tw])

        # ---- phase A: hT[i] = silu(W1[i]^T x + b1[i]) ----
        h = h_pool.tile([128, NI * TC], BF16, tag="h")
        for i in range(NI):
            ph = ph_pool.tile([128, TC], F32, tag="ph")
            for k in range(KH):
                nc.tensor.matmul(
                    ph[:, :tw],
                    w1_sb[:, i * WS + k * 128:i * WS + (k + 1) * 128],
                    xs[:, k * TC:k * TC + tw],
                    start=(k == 0),
                    stop=(k == KH - 1),
                )
            nc.scalar.activation(
                h[:, i * TC:i * TC + tw], ph[:, :tw], AF.Silu,
                bias=b1_sb[:, i:i + 1],
            )

        # ---- phase B: yT[hb] = sum_j W2[j][hb]^T hT[j]; +b2, *w, out ----
        for hb in range(NH):
            py = py_pool.tile([128, TC], F32, tag="py")
            for j in range(NI):
                nc.tensor.matmul(
                    py[:, :tw],
                    w2_sb[:, j * H + hb * 128:j * H + (hb + 1) * 128],
                    h[:, j * TC:j * TC + tw],
                    start=(j == 0),
                    stop=(j == NI - 1),
                )
            ys = ys_pool.tile([128, TC], F32, tag="ys")
            nc.scalar.activation(
                ys[:, :tw], py[:, :tw], AF.Identity, bias=b2_sb[:, hb:hb + 1],
            )
            nc.vector.tensor_tensor(
                ys[:, :tw], ys[:, :tw], wb_sb[:, t0:t0 + tw], op=ALU.mult
            )
            nc.sync.dma_start(yt[hb][:, t0:t0 + tw], ys[:, :tw])


def _route_host(xf, Wr):
    """f32 router matching the jax reference: softmax probs, stable top-2,
    renormalized combine weights."""
    logits = xf @ Wr
    m = logits.max(-1, keepdims=True)
    e = np.exp(logits - m)
    probs = e / e.sum(-1, keepdims=True)
    sel = np.argsort(-probs, axis=-1, kind="stable")[:, :TOP_K]
    rw = np.take_along_axis(probs, sel, axis=-1)
    rw = rw / rw.sum(-1, keepdims=True)
    return sel, rw


def make_in_maps(x, Wr, W1, b1, W2, b2):
    x = np.ascontiguousarray(np.asarray(x, dtype=np.float32))
    Wr = np.asarray(Wr, dtype=np.float32)
    W1 = np.asarray(W1, dtype=np.float32)
    b1 = np.asarray(b1, dtype=np.float32)
    W2 = np.asarray(W2, dtype=np.float32)
    b2 = np.asarray(b2, dtype=np.float32)

    xf = x.reshape(T, H)
    sel, rw = _route_host(xf, Wr)

    idx = [np.nonzero((sel == c).any(-1))[0] for c in range(E)]
    cap = max(len(ix) for ix in idx)
    cap = max(256, -(-cap // 128) * 128)

    in_maps = []
    for c in range(E):
        ix = idx[c]
        xgT = np.zeros((H, cap), BF16NP)
        xgT[:, :len(ix)] = xf[ix].T.astype(BF16NP)
        # combine weight of expert c for each routed token, broadcast to
        # all 128 partitions host-side (1.1MB DMA, zero device work)
        wc = np.where(sel[ix, 0] == c, rw[ix, 0], rw[ix, 1]).astype(np.float32)
        wbr = np.zeros((cap,), np.float32)
        wbr[:len(ix)] = wc
        wbb = np.ascontiguousarray(np.broadcast_to(wbr, (128, cap)))
        # w1 sbuf layout: [i-tile][p, k*128+f] = W1[k*128+p, i*128+f]
        w1r = np.ascontiguousarray(
            W1[c].reshape(KH, 128, NI, 128).transpose(2, 1, 0, 3)
            .reshape(NI, 128, KH * 128)
        ).astype(BF16NP)
        in_maps.append({
            "xg": np.ascontiguousarray(xgT.reshape(KH, 128, cap)),
            "w1": w1r,
            "w2": np.ascontiguousarray(W2[c].reshape(NI, 128, H)).astype(BF16NP),
            "wb": wbb,
            "b1r": np.ascontiguousarray(b1[c].reshape(NI, 128).T),
            "b2r": np.ascontiguousarray(b2[c].reshape(NH, 128).T),
        })
    return in_maps, idx, cap


def unshard(results, idx, cap):
    out = np.zeros((T, H), np.float32)
    for c in range(E):
        ix = idx[c]
        yc = results[c]["yt"].reshape(H, cap)
        out[ix] += yc.T[:len(ix)]
    return out.reshape(B, S, H)


def kernel_ex(x, Wr, W1, b1, W2, b2, trace=False):
    in_maps, idx, cap = make_in_maps(x, Wr, W1, b1, W2, b2)
    nc = _build_nc(cap)
    try:
        res = bass_utils.run_bass_kernel_spmd(
            nc, in_maps, core_ids=list(range(NCORES)), trace=trace
        )
    except ModuleNotFoundError:
        # no axon NTFF profile hook in this container -> run untraced
        res = bass_utils.run_bass_kernel_spmd(
            nc, in_maps, core_ids=list(range(NCORES)), trace=False
        )
    return unshard(res.results, idx, cap), res


def kernel(**inputs):
    out, _ = kernel_ex(**inputs)
    return out


# revision 29
# speedup vs baseline: 1.6261x; 1.0312x over previous
"""MoE top-2 routing kernel for Trainium2, expert-parallel across 8 NeuronCores.

Strategy (per sharding_hint, with a load-balance twist):
  - Host computes the router (softmax + top-2 + combine weights) in f32
    numpy (selection matches the jax reference; the router is 0.05% of
    total FLOPs) and builds the token->expert dispatch.
  - Experts are split across CORE PAIRS: the 4 heaviest experts (slot A)
    pair with the 4 lightest (slot B); core 2p takes i-tiles 0..15 of
    both its experts' W1/W2, core 2p+1 takes i-tiles 16..31. Each core
    runs two sub-kernels (slot A tokens, then slot B tokens) and writes
    partial yT = (silu(x@W1h + b1h) @ W2h) * w; the host adds the two
    halves and the (w * b2) term. This balances PE work: per-core rows
    scale with capA+capB (= max heavy + max light load) instead of
    2*max(all loads), and the lighter slot usually avoids a tail chunk.
  - All matmuls bf16 (1 cycle/row on PE); rel err ~3.4e-3 (gate 2e-2).

Per sub-kernel device structure (hardware-measured rationale in
sibling notes: a repeated LDWEIGHTS costs ~6ns vs ~29ns fresh; chunk
groups share each weight load):
  - x resident, W2-half resident, W1-half streamed in 2KB tiles.
  - Token chunks grouped (512,512)(+tail folded as a triple member);
    within a group each weight tile is loaded once, matmuls back-to-back.
  - Phase A: hT[i] = silu(W1[i]^T x + b1[i]), k-chain of 8 per chunk;
    phase B: yT[hb] = sum_j W2[j][hb]^T hT[j], j-chain of 16; epilogue
    (*combine-weight) on DVE; +b2 applied host-side.
  - x/y DMAs on the Activation engine's HWDGE queue, weights on sync.

Measured (hardware For_i loop differencing, 8 cores): ~570us/iter
steady state (single-expert-per-core version: ~585us; v1 baseline:
932us). bf16 row roofline for the balanced inventory is ~435us; the
rest is LDWEIGHTS/psum-chain turnaround (~40us), unexplained in-stream
overhead (~45us), and cross-engine contention (~45us).
"""

import numpy as np
import ml_dtypes

import concourse.bacc as bacc
import concourse.tile as tile
import concourse.mybir as mybir
from concourse import bass_utils

BF16NP = ml_dtypes.bfloat16
F32 = mybir.dt.float32
BF16 = mybir.dt.bfloat16
AF = mybir.ActivationFunctionType
ALU = mybir.AluOpType

B, S, H, I, E = 4, 2048, 1024, 4096, 8
T = B * S
TOP_K = 2
NCORES = 8
TC = 512            # token chunk (= max f32 PSUM bank width)
KH = H // 128       # 8  k-tiles over H (contraction of matmul 1)
NI = I // 128       # 32 i-tiles over I
NIS = NI // 2       # 16 i-tiles per core (expert split across a pair)
NH = H // 128       # 8  output h-tiles
WS = KH * 128


def _chunks(cap):
    out, t0 = [], 0
    while t0 < cap:
        tw = min(TC, cap - t0)
        out.append((t0, tw))
        t0 += tw
    return out


def _groups(cap):
    """Chunks grouped so consecutive matmuls share one weight load
    (LDWEIGHTS amortization): pairs of 512, with a small trailing chunk
    folded into the last group as a triple."""
    chunks = _chunks(cap)
    groups = [chunks[i:i + 2] for i in range(0, len(chunks), 2)]
    if len(groups) >= 2 and len(groups[-1]) == 1 and groups[-1][0][1] <= 256:
        groups[-2].extend(groups.pop())
    return groups


def _build_nc(capA, capB, loop_n=None, reps=1):
    nc = bacc.Bacc(
        "TRN2",
        target_bir_lowering=False,
        debug=False,
        enable_asserts=False,
        num_devices=NCORES,
    )
    io = {}
    for s, cap in (("a", capA), ("b", capB)):
        io[f"xg{s}"] = nc.dram_tensor(
            f"xg{s}", [KH, 128, cap], BF16, kind="ExternalInput").ap()
        io[f"w1{s}"] = nc.dram_tensor(
            f"w1{s}", [NIS, 128, WS], BF16, kind="ExternalInput").ap()
        io[f"w2{s}"] = nc.dram_tensor(
            f"w2{s}", [NIS, 128, H], BF16, kind="ExternalInput").ap()
        io[f"wb{s}"] = nc.dram_tensor(
            f"wb{s}", [128, cap], F32, kind="ExternalInput").ap()
        io[f"yt{s}"] = nc.dram_tensor(
            f"yt{s}", [NH, 128, cap], F32, kind="ExternalOutput").ap()
    b1r = nc.dram_tensor("b1r", [128, 2 * NIS], F32, kind="ExternalInput").ap()

    with tile.TileContext(nc) as tc:
        with (
            tc.tile_pool(name="consts", bufs=1) as cpool,
            tc.tile_pool(name="w1p", bufs=6) as w1_pool,
            tc.tile_pool(name="w2p", bufs=1) as w2_pool,
            tc.tile_pool(name="xsp", bufs=1) as xs_pool,
            tc.tile_pool(name="hp", bufs=1) as h_pool,
            tc.tile_pool(name="ysp", bufs=4) as ys_pool,
            tc.tile_pool(name="php", bufs=4, space="PSUM") as ph_pool,
            tc.tile_pool(name="pyp", bufs=4, space="PSUM") as py_pool,
        ):
            b1_sb = cpool.tile([128, 2 * NIS], F32)
            nc.sync.dma_start(b1_sb[:], b1r[:, :])
            wb_sbs = {}
            for s, cap in (("a", capA), ("b", capB)):
                wb_sbs[s] = cpool.tile([128, cap], F32, name=f"wb{s}_sb")
                nc.sync.dma_start(wb_sbs[s][:], io[f"wb{s}"][:, :])

            import contextlib
            env = dict(locals())
            loop_cm = (
                tc.For_i(0, loop_n, 1, hint_engines=(mybir.EngineType.PE,))
                if loop_n else contextlib.nullcontext()
            )
            with loop_cm:
                for _ in range(reps):
                    for si, (s, cap) in enumerate((("a", capA), ("b", capB))):
                        _emit_sub(nc, s, si, cap, env)

    nc.compile()
    return nc


def _emit_sub(nc, s, si, cap, env):
    io = env["io"]
    xg, w1, w2, yt = (io[f"xg{s}"], io[f"w1{s}"], io[f"w2{s}"], io[f"yt{s}"])
    wb_sb = env["wb_sbs"][s]
    b1_sb = env["b1_sb"][:, si * NIS:(si + 1) * NIS]
    w1_pool, w2_pool = env["w1_pool"], env["w2_pool"]
    xs_pool, h_pool, ys_pool = env["xs_pool"], env["h_pool"], env["ys_pool"]
    ph_pool, py_pool = env["ph_pool"], env["py_pool"]

    groups = _groups(cap)

    # x resident (chunk-major so the first group's slab lands first);
    # x/y ride the Activation engine's HWDGE queue so weight prefetch
    # (sync queue) is never head-of-line blocked behind them
    xall = xs_pool.tile([128, KH * cap], BF16, tag=f"xall{s}",
                        name=f"xall{s}")
    for g in groups:
        g0 = g[0][0]
        gw = sum(tw for _, tw in g)
        for k in range(KH):
            nc.scalar.dma_start(
                xall[:, k * cap + g0:k * cap + g0 + gw], xg[k][:, g0:g0 + gw]
            )

    w2_sb = w2_pool.tile([128, NIS * H], BF16, tag=f"w2{s}", name=f"w2{s}_sb")

    for gi, g in enumerate(groups):
        gw = sum(tw for _, tw in g)
        offs = []  # (off-in-group, t0, tw)
        o = 0
        for (t0, tw) in g:
            offs.append((o, t0, tw))
            o += tw

        # ---- phase A: hT[i] = silu(W1[i]^T x + b1[i]), weight load
        # shared across the group's chunks ----
        h = h_pool.tile([128, NIS * gw], BF16, tag="h", name="h")
        for i in range(NIS):
            w1t = w1_pool.tile([128, WS], BF16, tag="w1t")
            nc.sync.dma_start(w1t[:], w1[i])
            phs = [ph_pool.tile([128, TC], F32, tag="ph", name=f"ph{ci}")
                   for ci in range(len(g))]
            for k in range(KH):
                for ci, (o, t0, tw) in enumerate(offs):
                    nc.tensor.matmul(
                        phs[ci][:, :tw],
                        w1t[:, k * 128:(k + 1) * 128],
                        xall[:, k * cap + t0:k * cap + t0 + tw],
                        start=(k == 0),
                        stop=(k == KH - 1),
                    )
            for ci, (o, t0, tw) in enumerate(offs):
                nc.scalar.activation(
                    h[:, i * gw + o:i * gw + o + tw], phs[ci][:, :tw], AF.Silu,
                    bias=b1_sb[:, i:i + 1],
                )

        # W2-half DMA after the first group's phase A: the queue has
        # drained the small x/w1 prefixes and W2 lands long before B0.
        if gi == 0:
            for j in range(NIS):
                nc.sync.dma_start(w2_sb[:, j * H:(j + 1) * H], w2[j])

        # ---- phase B: yT[hb] = sum_j W2[j][hb]^T hT[j]; *w, out ----
        for hb in range(NH):
            pys = [py_pool.tile([128, TC], F32, tag="py", name=f"py{ci}")
                   for ci in range(len(g))]
            for j in range(NIS):
                for ci, (o, t0, tw) in enumerate(offs):
                    nc.tensor.matmul(
                        pys[ci][:, :tw],
                        w2_sb[:, j * H + hb * 128:j * H + (hb + 1) * 128],
                        h[:, j * gw + o:j * gw + o + tw],
                        start=(j == 0),
                        stop=(j == NIS - 1),
                    )
            for ci, (o, t0, tw) in enumerate(offs):
                ys = ys_pool.tile([128, TC], F32, tag="ys", name="ys")
                # epilogue on DVE only; b2 is applied host-side (the two
                # half-partials would double-add it on device)
                nc.vector.tensor_tensor(
                    ys[:, :tw], pys[ci][:, :tw], wb_sb[:, t0:t0 + tw],
                    op=ALU.mult,
                )
                nc.scalar.dma_start(yt[hb][:, t0:t0 + tw], ys[:, :tw])


def _route_host(xf, Wr):
    """f32 router matching the jax reference: softmax probs, stable top-2,
    renormalized combine weights."""
    logits = xf @ Wr
    m = logits.max(-1, keepdims=True)
    e = np.exp(logits - m)
    probs = e / e.sum(-1, keepdims=True)
    sel = np.argsort(-probs, axis=-1, kind="stable")[:, :TOP_K]
    rw = np.take_along_axis(probs, sel, axis=-1)
    rw = rw / rw.sum(-1, keepdims=True)
    return sel, rw


def _round_cap(n):
    # multiple of 8 keeps every per-partition DMA line 4B-aligned (bf16)
    return max(256, -(-n // 8) * 8)


def make_in_maps(x, Wr, W1, b1, W2, b2):
    x = np.ascontiguousarray(np.asarray(x, dtype=np.float32))
    Wr = np.asarray(Wr, dtype=np.float32)
    W1 = np.asarray(W1, dtype=np.float32)
    b1 = np.asarray(b1, dtype=np.float32)
    W2 = np.asarray(W2, dtype=np.float32)
    b2 = np.asarray(b2, dtype=np.float32)

    xf = x.reshape(T, H)
    sel, rw = _route_host(xf, Wr)

    idx = [np.nonzero((sel == c).any(-1))[0] for c in range(E)]
    wfull = [
        np.where(sel[idx[c], 0] == c, rw[idx[c], 0], rw[idx[c], 1])
        .astype(np.float32) for c in range(E)
    ]
    # slot A = 4 heaviest experts, slot B = 4 lightest; pair p runs
    # (slotA[p], slotB[p]) split across cores 2p (i-tiles 0..15) and
    # 2p+1 (i-tiles 16..31)
    order = sorted(range(E), key=lambda c: -len(idx[c]))
    slotA, slotB = order[:E // 2], order[E // 2:]
    capA = _round_cap(max(len(idx[c]) for c in slotA))
    capB = _round_cap(max(len(idx[c]) for c in slotB))

    def pack(c, cap, half):
        ix = idx[c]
        xgT = np.zeros((H, cap), BF16NP)
        xgT[:, :len(ix)] = xf[ix].T.astype(BF16NP)
        wbr = np.zeros((cap,), np.float32)
        wbr[:len(ix)] = wfull[c]
        isl = slice(half * NIS * 128, (half + 1) * NIS * 128)
        w1r = np.ascontiguousarray(
            W1[c][:, isl].reshape(KH, 128, NIS, 128).transpose(2, 1, 0, 3)
            .reshape(NIS, 128, WS)
        ).astype(BF16NP)
        return {
            "xg": np.ascontiguousarray(xgT.reshape(KH, 128, cap)),
            "w1": w1r,
            "w2": np.ascontiguousarray(
                W2[c][isl].reshape(NIS, 128, H)).astype(BF16NP),
            "wb": np.ascontiguousarray(np.broadcast_to(wbr, (128, cap))),
            "b1": np.ascontiguousarray(b1[c][isl].reshape(NIS, 128).T),
        }

    in_maps = []
    for core in range(NCORES):
        p, half = core // 2, core % 2
        pa = pack(slotA[p], capA, half)
        pb = pack(slotB[p], capB, half)
        in_maps.append({
            "xga": pa["xg"], "w1a": pa["w1"], "w2a": pa["w2"], "wba": pa["wb"],
            "xgb": pb["xg"], "w1b": pb["w1"], "w2b": pb["w2"], "wbb": pb["wb"],
            "b1r": np.ascontiguousarray(
                np.concatenate([pa["b1"], pb["b1"]], axis=1)),
        })
    meta = dict(idx=idx, wfull=wfull, slotA=slotA, slotB=slotB,
                capA=capA, capB=capB, b2=b2)
    return in_maps, meta


def unshard(results, meta):
    out = np.zeros((T, H), np.float32)
    for p in range(NCORES // 2):
        for s, key, cap in (("a", "yta", meta["capA"]), ("b", "ytb", meta["capB"])):
            c = (meta["slotA"] if s == "a" else meta["slotB"])[p]
            ix = meta["idx"][c]
            yc = (results[2 * p][key].reshape(H, cap)
                  + results[2 * p + 1][key].reshape(H, cap))
            out[ix] += yc.T[:len(ix)]
            # b2 applied host-side, scaled by the combine weight
            out[ix] += np.outer(meta["wfull"][c], meta["b2"][c])
    return out.reshape(B, S, H)


def kernel_ex(x, Wr, W1, b1, W2, b2, trace=False):
    in_maps, meta = make_in_maps(x, Wr, W1, b1, W2, b2)
    nc = _build_nc(meta["capA"], meta["capB"])
    try:
        res = bass_utils.run_bass_kernel_spmd(
            nc, in_maps, core_ids=list(range(NCORES)), trace=trace
        )
    except ModuleNotFoundError:
        # no axon NTFF profile hook in this container -> run untraced
        res = bass_utils.run_bass_kernel_spmd(
            nc, in_maps, core_ids=list(range(NCORES)), trace=False
        )
    return unshard(res.results, meta), res


def kernel(**inputs):
    out, _ = kernel_ex(**inputs)
    return out
